# revision 12
# baseline (speedup 1.0000x reference)
"""GATv2 attention head (single head) on 8 Trainium2 NeuronCores.

Math: with h = x @ W1^T + b, z = leaky_relu(h), s2 = z@a2, the GATv2 segment
softmax over src makes the s1[src] term cancel (constant per segment), so
with p = exp(s2):

    out[i] = sum_{j in N(i) ∪ {i}} p[j] * h[j] / sum_{j} p[j]

i.e. a weighted average of h rows with per-node weights p[j].  What remains
is one segment-sum over edges of gathered rows g[j] = [p[j]*h[j], p[j]].

Sharding: edges are bucketed by src range (12500 nodes per core).  Each core
computes g for its range, AllGathers the full g table (bf16, 256B row
stride), then gathers g[dst] for its edges with the SWDGE dma_gather ucode
op (int16 indices => the 102400-row table is split into 4 quarters of 25600
rows) and segment-sums with a strided DVE reduction.  To keep the reduce
static, every node gets D slots per quarter where D (its "class") is the
smallest allowed value >= max over quarters of its per-quarter edge count
(self loop included, placed in the least-loaded quarter); pad slots point
at an all-zero table row.  The 4 quarter partials for a node land at the
same (partition, group) position, so one 5D reduce sums over both the slot
and quarter axes at once.
"""
import sys
sys.path.insert(0, '/opt/trn_rl_repo')

import numpy as np
import jax
from jax.sharding import Mesh, PartitionSpec
from jax.experimental.shard_map import shard_map

import concourse.bass as bass
import concourse.bacc as bacc
import concourse.mybir as mybir
import concourse.tile as tile
from concourse.bass import exact_div
from concourse.masks import make_identity
from concourse import bass2jax
from concourse.bass2jax import _bass_exec_p, install_neuronx_cc_hook

F32 = mybir.dt.float32
BF16 = mybir.dt.bfloat16
I16 = mybir.dt.int16

NCORES = 8
N = 100000
DIN = 128
DOUT = 32
SLOPE = 0.2
NPC = N // NCORES            # nodes per core (12500)
NODE_PAD = 12800             # padded nodes per core
GF = DOUT + 2                # stored row: [p*h (32), p, 0]
GSTRIDE = 128                # bf16 elements per table row (256B)
NQ = 4                       # index-range quarters (int16 limit)
QROWS = NCORES * NODE_PAD // NQ   # 25600 rows per quarter
ZLOCAL = NODE_PAD - 1        # per-core always-zero g row
QZERO = NODE_PAD + ZLOCAL    # quarter-local zero row (25599), in every quarter
MMCHUNK = 512
NCHUNKS = NODE_PAD // MMCHUNK
GMAXC = 32                   # max slot columns (4096 idx) per dma_gather call

# degree classes: D slots per node per quarter; cols per chunk = npp*D must
# be a multiple of GCOLS(8)
CLASS_NPP = {2: 64, 4: 32, 6: 20, 8: 16, 10: 12, 12: 10, 14: 8, 16: 8,
             20: 6, 24: 5, 28: 4, 32: 4, 40: 3, 48: 2, 64: 2, 96: 1, 128: 1}
CLASS_D = sorted(CLASS_NPP)


def dma_gather_raw(nc, out_ap, in_ap, idxs_ap, num_idxs, elem_size,
                   elem_step, single_packet=False):
    """bass.dma_gather minus the elem%256 assert (only the row *stride*
    must be a 256B multiple for the ucode)."""
    assert idxs_ap.dtype == I16
    assert in_ap.dtype == out_ap.dtype
    assert in_ap.ap[0][0] == elem_step
    stride_bytes_256 = exact_div(elem_step * mybir.dt.size(in_ap.dtype), 256)
    eng = nc.gpsimd
    _in_ap = eng.lower_ap_dma(in_ap, for_custom_bir_dma=True)
    _idxs_ap = eng.lower_ap(idxs_ap)
    _out_ap = eng.lower_ap(out_ap)
    return eng.add_instruction(
        mybir.InstDMAGatherAnt(
            name=nc.get_next_instruction_name(),
            ins=[*_in_ap, _idxs_ap,
                 eng.lower_val_access(eng.to_reg(num_idxs))],
            outs=[_out_ap],
            transpose=False, num_idxs=num_idxs, elem_size=elem_size,
            stride_bytes_256=stride_bytes_256, gen_mode=0,
            single_packet=single_packet, queue_num=0, sbuf_tokens_per_rank=0,
            sbuf_free_dim_per_rank=0, sbuf_free_dim_pad_per_rank=0,
            sbuf_byte_offset=0))


def _host_shard(x, edge_index):
    """Bucket edges by src range; build per-core quarter slot indices."""
    src = np.asarray(edge_index[0]).astype(np.int64)
    dst = np.asarray(edge_index[1]).astype(np.int64)
    x = np.asarray(x)
    dstp = (dst // NPC) * NODE_PAD + (dst % NPC)   # padded-global row

    cores = []
    for c in range(NCORES):
        sel = np.flatnonzero(src // NPC == c)
        s_l = src[sel] - c * NPC
        d_p = dstp[sel]
        q_d = d_p // QROWS
        # per (node, quarter) counts
        need = np.zeros((NPC, NQ), np.int64)
        np.add.at(need, (s_l, q_d), 1)
        # self loop: its table row lives in quarter c//2
        self_q = np.full(NPC, c // 2, np.int64)
        need[np.arange(NPC), self_q] += 1
        dmax = need.max(axis=1)
        cls = np.full(NPC, -1, np.int64)
        for D in CLASS_D:
            m = (cls == -1) & (dmax <= D)
            cls[m] = D
        assert (cls > 0).all(), "degree exceeds largest class"
        cores.append(dict(c=c, s_l=s_l, d_p=d_p, q_d=q_d, need=need,
                          self_q=self_q, cls=cls))

    # shared chunk structure (same on all cores for SPMD)
    chunk_counts = {}
    for D in CLASS_D:
        mx = max(int((cd['cls'] == D).sum()) for cd in cores)
        npp = CLASS_NPP[D]
        chunk_counts[D] = -(-mx // (128 * npp)) if mx else 0

    # plan: one entry per class-chunk
    plan = []
    g0 = 0          # node-group offset
    col0 = 0        # slot-column offset (per quarter)
    for D in CLASS_D:
        npp = CLASS_NPP[D]
        for _ in range(chunk_counts[D]):
            plan.append(dict(D=D, npp=npp, cols=npp * D, g0=g0, col0=col0))
            g0 += npp
            col0 += npp * D
    qtot, ctot = g0, col0
    assert qtot > 0
    tot_idx = NQ * ctot * 128 * 8

    per_core_inputs = []
    perms = []
    for cd in cores:
        c, cls = cd['c'], cd['cls']
        # node -> (class-chunk, position)
        col_of_node = np.full(NPC, -1, np.int64)   # node's first slot column
        perm = np.full(qtot * 128, -1, np.int64)
        for D in CLASS_D:
            if chunk_counts[D] == 0:
                continue
            nodes = np.flatnonzero(cls == D)
            npp = CLASS_NPP[D]
            base_g = min(ch['g0'] for ch in plan if ch['D'] == D)
            base_col = min(ch['col0'] for ch in plan if ch['D'] == D)
            t = np.arange(len(nodes))
            chunk = t // (128 * npp)
            i_loc = t % (128 * npp)
            p = i_loc % 128
            j = i_loc // 128
            # groups are contiguous per class (chunks are consecutive)
            perm[(base_g + chunk * npp + j) * 128 + p] = nodes
            col_of_node[nodes] = base_col + chunk * npp * D + j * D
        node_p = np.full(NPC, -1, np.int64)
        m = np.flatnonzero(perm >= 0)
        node_p[perm[m]] = m % 128

        # slot matrix [4 quarters, ctot cols, 128 partitions] of quarter-local
        # rows, default = quarter zero row
        slots = np.full((NQ, ctot, 128), QZERO, np.int16)
        # self loops
        n_ar = np.arange(NPC)
        selfrow = (c * NODE_PAD + n_ar) % QROWS
        sq = cd['self_q']
        slots[sq, col_of_node[n_ar], node_p[n_ar]] = selfrow.astype(np.int16)
        # edges: rank within (node, quarter), offset +1 in self quarter
        s_l, d_p, q_d = cd['s_l'], cd['d_p'], cd['q_d']
        order = np.lexsort((q_d, s_l))
        ss, dd, qq = s_l[order], d_p[order], q_d[order]
        # rank of each sorted edge within its (node, quarter) run
        key_change = np.flatnonzero(
            (np.diff(ss) != 0) | (np.diff(qq) != 0)) + 1
        starts = np.zeros(len(ss), np.int64)
        starts[key_change] = key_change
        np.maximum.accumulate(starts, out=starts)
        rank = np.arange(len(ss)) - starts
        rank = rank + (qq == sq[ss])       # slot 0 = self loop there
        col = col_of_node[ss] + rank
        slots[qq, col, node_p[ss]] = (dd % QROWS).astype(np.int16)

        # reorder into the per-call [16, 64] wrapped layout:
        # input stream order: for chunk t, quarter q, octet o: block
        blocks = []
        for ch in plan:
            cols = ch['cols']
            pieces = [GMAXC] * (cols // GMAXC)
            if cols % GMAXC:
                pieces.append(cols % GMAXC)
            for q in range(NQ):
                coff = 0
                for pc in pieces:
                    cs = slots[q, ch['col0'] + coff:
                               ch['col0'] + coff + pc, :]
                    arr = cs.reshape(-1)                 # [pc*128] c-major
                    arr = arr.reshape(pc * 8, 16).T      # idx i at [i%16,i//16]
                    arr = np.tile(arr, (8, 1))           # [128, pc*8]
                    blocks.append(arr.reshape(-1))
                    coff += pc
        slot_stream = np.concatenate(blocks).astype(np.int16)
        assert slot_stream.size == tot_idx, (slot_stream.size, tot_idx)

        xT = np.zeros((DIN, NODE_PAD), np.float32)
        xT[:, :NPC] = x[c * NPC:(c + 1) * NPC].T
        per_core_inputs.append({"xT": xT, "slots": slot_stream})
        perms.append(perm)

    return plan, qtot, ctot, tot_idx, per_core_inputs, perms


def _build_program(plan, qtot, ctot, tot_idx):
    nc = bacc.Bacc("TRN2", target_bir_lowering=False, debug=False,
                   num_devices=NCORES)
    xT_d = nc.dram_tensor("xT", [DIN, NODE_PAD], F32, kind="ExternalInput")
    w_d = nc.dram_tensor("w1t", [DIN, DOUT], F32, kind="ExternalInput")
    b_d = nc.dram_tensor("bias", [DOUT, 1], F32, kind="ExternalInput")
    a2_d = nc.dram_tensor("a2", [DOUT, GF], F32, kind="ExternalInput")
    slots_d = nc.dram_tensor("slots", [tot_idx], I16, kind="ExternalInput")
    out_d = nc.dram_tensor("out", [qtot * 128, DOUT], F32,
                           kind="ExternalOutput")

    with tile.TileContext(nc) as tc:
        with tc.tile_pool(name="const", bufs=1) as constp, \
             tc.tile_pool(name="xp", bufs=3) as xp, \
             tc.tile_pool(name="work", bufs=3) as work, \
             tc.tile_pool(name="big", bufs=1) as big, \
             tc.tile_pool(name="gbuf", bufs=1) as gbuf, \
             tc.tile_pool(name="ibuf", bufs=2) as ibuf, \
             tc.tile_pool(name="ps", bufs=2, space="PSUM") as ps, \
             tc.tile_pool(name="pst", bufs=2, space="PSUM") as pst, \
             tc.tile_pool(name="dram", bufs=1, space="DRAM") as dram:

            w_sb = constp.tile([DIN, DOUT], F32)
            nc.sync.dma_start(out=w_sb[:], in_=w_d[:, :])
            b_sb = constp.tile([DOUT, 1], F32)
            nc.sync.dma_start(out=b_sb[:], in_=b_d[:, :])
            a2_sb = constp.tile([DOUT, GF], F32)
            nc.sync.dma_start(out=a2_sb[:], in_=a2_d[:, :])
            ident = constp.tile([128, 128], F32)
            make_identity(nc, ident[:])

            # ---- node phase: gT[f, n] = [p*h; p; 0] ----
            gT = big.tile([GF, NODE_PAD], F32, tag="gT")
            # feature 33 stays zero (padding); rows 32-33 zeroed up front
            # (partition slices must be 32-aligned)
            nc.vector.memset(gT[DOUT:GF, :], 0.0)
            for t in range(NCHUNKS):
                cs = slice(t * MMCHUNK, (t + 1) * MMCHUNK)
                xt = xp.tile([DIN, MMCHUNK], F32)
                nc.sync.dma_start(out=xt[:], in_=xT_d[:, cs])
                hps = ps.tile([DOUT, MMCHUNK], F32, space="PSUM")
                nc.tensor.matmul(hps[:], lhsT=w_sb[:], rhs=xt[:],
                                 start=True, stop=True)
                h_sb = work.tile([DOUT, MMCHUNK], F32, tag="h")
                nc.vector.tensor_tensor(
                    out=h_sb[:], in0=hps[:],
                    in1=b_sb[:].to_broadcast([DOUT, MMCHUNK]),
                    op=mybir.AluOpType.add)
                z_sb = work.tile([DOUT, MMCHUNK], F32, tag="z")
                nc.vector.scalar_tensor_tensor(
                    out=z_sb[:], in0=h_sb[:], scalar=SLOPE,
                    in1=h_sb[:], op0=mybir.AluOpType.mult,
                    op1=mybir.AluOpType.max)
                sps = ps.tile([GF, MMCHUNK], F32, space="PSUM", tag="s2")
                nc.tensor.matmul(sps[:], lhsT=a2_sb[:], rhs=z_sb[:],
                                 start=True, stop=True)
                p_sb = work.tile([GF, MMCHUNK], F32, tag="p")
                nc.scalar.activation(out=p_sb[:], in_=sps[:],
                                     func=mybir.ActivationFunctionType.Exp)
                nc.vector.tensor_tensor(
                    out=gT[0:DOUT, cs], in0=h_sb[:], in1=p_sb[0:DOUT, :],
                    op=mybir.AluOpType.mult)
                nc.vector.tensor_copy(out=gT[DOUT:DOUT + 1, cs],
                                      in_=p_sb[DOUT:DOUT + 1, :])
            # node ZLOCAL must be all zeros (pad-slot target)
            nc.vector.memset(gT[:, ZLOCAL:ZLOCAL + 1], 0.0)

            # ---- transpose gT -> node-major, cast to bf16 table ----
            ntile = NODE_PAD // 128
            g_sb = big.tile([128, ntile * GF], F32)
            for t in range(ntile):
                tp = pst.tile([128, GF], F32, space="PSUM")
                nc.tensor.transpose(
                    out=tp[:], in_=gT[:, t * 128:(t + 1) * 128],
                    identity=ident[:GF, :GF])
                nc.vector.tensor_copy(
                    out=g_sb[:, t * GF:(t + 1) * GF], in_=tp[:])

            g_loc = dram.tile([NODE_PAD, GSTRIDE], BF16)
            nc.gpsimd.dma_start(
                out=g_loc[:, 0:GF].rearrange("(t p) f -> p t f", p=128),
                in_=g_sb[:].rearrange("p (t f) -> p t f", f=GF))
            zfill = work.tile([128, GSTRIDE - GF], BF16, tag="zf")
            nc.vector.memset(zfill[:], 0.0)
            nc.sync.dma_start(
                out=g_loc[:, GF:GSTRIDE].rearrange("(t p) f -> p t f", p=128),
                in_=bass.AP(zfill[:].tensor, 0,
                            [[GSTRIDE - GF, 128], [0, NODE_PAD // 128],
                             [1, GSTRIDE - GF]]))
            g_full = dram.tile([NCORES * NODE_PAD, GSTRIDE], BF16,
                               addr_space="Shared")
            nc.gpsimd.collective_compute(
                "AllGather", mybir.AluOpType.bypass,
                ins=[g_loc[:].opt()], outs=[g_full[:].opt()],
                replica_groups=[list(range(NCORES))])

            # ---- edge phase ----
            outbig = big.tile([128, qtot * GF], F32, tag="gT")
            ob = outbig[:].rearrange("p (q f) -> p q f", f=GF)
            ioff = 0
            for ch in plan:
                D, npp, cols, g0 = ch['D'], ch['npp'], ch['cols'], ch['g0']
                B = gbuf.tile([128, NQ * cols * GF], BF16, tag="B")
                pieces = [GMAXC] * (cols // GMAXC)
                if cols % GMAXC:
                    pieces.append(cols % GMAXC)
                idx = ibuf.tile([128, NQ * cols * 8], I16, tag="idx")
                foff = 0
                for q in range(NQ):
                    qsl = g_full[q * QROWS:(q + 1) * QROWS, 0:GF]
                    coff = 0
                    for pc in pieces:
                        blk = pc * 8
                        nc.sync.dma_start(
                            out=idx[:, foff:foff + blk],
                            in_=bass.AP(slots_d[:].tensor, ioff,
                                        [[blk, 128], [1, blk]]))
                        lo = (q * cols + coff) * GF
                        dst = B[:, lo:lo + pc * GF].rearrange(
                            "p (c f) -> p c f", f=GF)
                        dma_gather_raw(nc, dst, qsl,
                                       idx[:, foff:foff + blk],
                                       pc * 128, GF, GSTRIDE)
                        ioff += 128 * blk
                        foff += blk
                        coff += pc
                # reduce over (slot d, quarter q): 5D AP, X=d then Y=q
                inap = B[:].rearrange("p (q j d f) -> p j f q d",
                                      q=NQ, j=npp, d=D, f=GF)
                nc.vector.reduce_sum(out=ob[:, g0:g0 + npp, :], in_=inap,
                                     axis=mybir.AxisListType.XY)

            # ---- normalize and write out ----
            dt_ = work.tile([128, qtot], F32, tag="den")
            nc.vector.tensor_scalar_add(out=dt_[:], in0=ob[:, :, DOUT],
                                        scalar1=1e-30)
            rec = work.tile([128, qtot], F32, tag="rec")
            nc.vector.reciprocal(out=rec[:], in_=dt_[:])
            nc.vector.tensor_tensor(
                out=ob[:, :, 0:DOUT], in0=ob[:, :, 0:DOUT],
                in1=rec[:].to_broadcast([128, qtot, DOUT]),
                op=mybir.AluOpType.mult)
            nc.sync.dma_start(
                out=out_d[:, :].rearrange("(q p) f -> p q f", p=128),
                in_=ob[:, :, 0:DOUT])

    nc.compile()
    return nc


class _Runner:
    """shard_map-jitted executor (mirrors bass2jax.run_bass_via_pjrt)."""

    def __init__(self, nc, n_cores):
        install_neuronx_cc_hook()
        self.n_cores = n_cores
        partition_name = (nc.partition_id_tensor.name
                          if nc.partition_id_tensor else None)
        in_names, out_names, out_avals, zero_outs = [], [], [], []
        for alloc in nc.m.functions[0].allocations:
            if not isinstance(alloc, mybir.MemoryLocationSet):
                continue
            name = alloc.memorylocations[0].name
            if alloc.kind == "ExternalInput":
                if name != partition_name:
                    in_names.append(name)
            elif alloc.kind == "ExternalOutput":
                out_names.append(name)
                shape = tuple(alloc.tensor_shape)
                dtype = mybir.dt.np(alloc.dtype)
                out_avals.append(jax.core.ShapedArray(shape, dtype))
                zero_outs.append(np.zeros(shape, dtype))
        self.in_names = in_names
        self.out_names = out_names
        self.out_avals = out_avals
        self.zero_outs = zero_outs
        n_params = len(in_names)
        self.n_params = n_params
        all_in = in_names + out_names
        if partition_name is not None:
            all_in.append(partition_name)
        donate = tuple(range(n_params, n_params + len(out_avals)))

        def _body(*args):
            operands = list(args)
            if partition_name is not None:
                operands.append(bass2jax.partition_id_tensor())
            outs = _bass_exec_p.bind(
                *operands, out_avals=tuple(out_avals),
                in_names=tuple(all_in), out_names=tuple(out_names),
                lowering_input_output_aliases=(),
                sim_require_finite=True, sim_require_nnan=True, nc=nc)
            return tuple(outs)

        devices = jax.devices()[:n_cores]
        mesh = Mesh(np.asarray(devices), ("core",))
        self._fn = jax.jit(
            shard_map(_body, mesh=mesh,
                      in_specs=(PartitionSpec("core"),) * (n_params +
                                                           len(out_avals)),
                      out_specs=(PartitionSpec("core"),) * len(out_names),
                      check_rep=False),
            donate_argnums=donate, keep_unused=True)

    def run(self, in_maps):
        per_core = [[np.asarray(m[n]) for n in self.in_names]
                    for m in in_maps]
        concat_in = [
            np.concatenate([per_core[c][i] for c in range(self.n_cores)],
                           axis=0)
            for i in range(self.n_params)
        ]
        concat_zeros = [
            np.zeros((self.n_cores * z.shape[0], *z.shape[1:]), z.dtype)
            for z in self.zero_outs
        ]
        out_arrs = self._fn(*concat_in, *concat_zeros)
        jax.block_until_ready(out_arrs)
        return [
            {name: np.asarray(out_arrs[i]).reshape(
                self.n_cores, *self.out_avals[i].shape)[c]
             for i, name in enumerate(self.out_names)}
            for c in range(self.n_cores)
        ]


_CACHE = {}


def _consts(W1_w, W1_b, a2_w):
    return {
        "w1t": np.ascontiguousarray(np.asarray(W1_w).T).astype(np.float32),
        "bias": np.asarray(W1_b).reshape(DOUT, 1).astype(np.float32),
        "a2": np.repeat(np.asarray(a2_w).reshape(DOUT, 1), GF,
                        axis=1).astype(np.float32),
    }


def _get_runner(plan, qtot, ctot, tot_idx):
    key = (tuple((ch['D'], ch['g0']) for ch in plan), qtot, ctot, tot_idx)
    if key not in _CACHE:
        nc = _build_program(plan, qtot, ctot, tot_idx)
        _CACHE[key] = (nc, _Runner(nc, NCORES))
    return _CACHE[key]


def kernel(x, edge_index, W1_w, W1_b, a1_w=None, a2_w=None):
    plan, qtot, ctot, tot_idx, per_core, perms = _host_shard(x, edge_index)
    nc, runner = _get_runner(plan, qtot, ctot, tot_idx)
    consts = _consts(W1_w, W1_b, a2_w)
    in_maps = [{**per_core[c], **consts} for c in range(NCORES)]
    results = runner.run(in_maps)
    out = np.empty((N, DOUT), np.float32)
    for c in range(NCORES):
        rows = results[c]["out"]
        perm = perms[c]
        valid = perm >= 0
        out[c * NPC + perm[valid]] = rows[valid]
    return out


# revision 13
# speedup vs baseline: 1.2361x; 1.2361x over previous
"""GATv2 attention head (single head) on 8 Trainium2 NeuronCores.

Math: with h = x @ W1^T + b, z = leaky_relu(h), s2 = z@a2, the GATv2 segment
softmax over src makes the s1[src] term cancel (constant per segment), so
with p = exp(s2):

    out[i] = sum_{j in N(i) ∪ {i}} p[j] * h[j] / sum_{j} p[j]

i.e. a weighted average of h rows with per-node weights p[j].  What remains
is one segment-sum over edges of gathered rows g[j] = [p[j]*h[j], p[j]].

Sharding: edges are bucketed by src range (12500 nodes per core).  Each core
computes g for its range, AllGathers the full g table (bf16, 256B row
stride), then gathers g[dst] for its edges with the SWDGE dma_gather ucode
op (int16 indices => the 102400-row table is split into 4 quarters of 25600
rows) and segment-sums with a strided DVE reduction.  To keep the reduce
static, every node gets D slots per quarter where D (its "class") is the
smallest allowed value >= max over quarters of its per-quarter edge count
(self loop included, placed in the least-loaded quarter); pad slots point
at an all-zero table row.  The 4 quarter partials for a node land at the
same (partition, group) position, so one 5D reduce sums over both the slot
and quarter axes at once.
"""
import sys
sys.path.insert(0, '/opt/trn_rl_repo')

import numpy as np
import jax
from jax.sharding import Mesh, PartitionSpec
from jax.experimental.shard_map import shard_map

import concourse.bass as bass
import concourse.bacc as bacc
import concourse.mybir as mybir
import concourse.tile as tile
from concourse.bass import exact_div
from concourse.masks import make_identity
from concourse import bass2jax
from concourse.bass2jax import _bass_exec_p, install_neuronx_cc_hook

F32 = mybir.dt.float32
BF16 = mybir.dt.bfloat16
I16 = mybir.dt.int16

NCORES = 8
N = 100000
DIN = 128
DOUT = 32
SLOPE = 0.2
NPC = N // NCORES            # nodes per core (12500)
NODE_PAD = 12800             # padded nodes per core
GF = DOUT + 2                # stored row: [p*h (32), p, 0]
GSTRIDE = 128                # bf16 elements per table row (256B)
NQ = 4                       # index-range quarters (int16 limit)
QROWS = NCORES * NODE_PAD // NQ   # 25600 rows per quarter
ZLOCAL = NODE_PAD - 1        # per-core always-zero g row
QZERO = NODE_PAD + ZLOCAL    # quarter-local zero row (25599), in every quarter
MMCHUNK = 512
NCHUNKS = NODE_PAD // MMCHUNK
GMAXC = 32                   # max slot columns (4096 idx) per dma_gather call

# degree classes: D slots per node per quarter; cols per chunk = npp*D must
# be a multiple of GCOLS(8)
CLASS_NPP = {2: 64, 4: 32, 6: 20, 8: 16, 10: 12, 12: 10, 14: 8, 16: 8,
             20: 6, 24: 5, 28: 4, 32: 4, 40: 3, 48: 2, 64: 2, 96: 1, 128: 1}
CLASS_D = sorted(CLASS_NPP)


def dma_gather_raw(nc, out_ap, in_ap, idxs_ap, num_idxs, elem_size,
                   elem_step, single_packet=False):
    """bass.dma_gather minus the elem%256 assert (only the row *stride*
    must be a 256B multiple for the ucode)."""
    assert idxs_ap.dtype == I16
    assert in_ap.dtype == out_ap.dtype
    assert in_ap.ap[0][0] == elem_step
    stride_bytes_256 = exact_div(elem_step * mybir.dt.size(in_ap.dtype), 256)
    eng = nc.gpsimd
    _in_ap = eng.lower_ap_dma(in_ap, for_custom_bir_dma=True)
    _idxs_ap = eng.lower_ap(idxs_ap)
    _out_ap = eng.lower_ap(out_ap)
    return eng.add_instruction(
        mybir.InstDMAGatherAnt(
            name=nc.get_next_instruction_name(),
            ins=[*_in_ap, _idxs_ap,
                 eng.lower_val_access(eng.to_reg(num_idxs))],
            outs=[_out_ap],
            transpose=False, num_idxs=num_idxs, elem_size=elem_size,
            stride_bytes_256=stride_bytes_256, gen_mode=0,
            single_packet=single_packet, queue_num=0, sbuf_tokens_per_rank=0,
            sbuf_free_dim_per_rank=0, sbuf_free_dim_pad_per_rank=0,
            sbuf_byte_offset=0))


def _host_shard(x, edge_index):
    """Bucket edges by src range; build per-core quarter slot indices."""
    src = np.asarray(edge_index[0]).astype(np.int64)
    dst = np.asarray(edge_index[1]).astype(np.int64)
    x = np.asarray(x)
    dstp = (dst // NPC) * NODE_PAD + (dst % NPC)   # padded-global row

    cores = []
    for c in range(NCORES):
        sel = np.flatnonzero(src // NPC == c)
        s_l = src[sel] - c * NPC
        d_p = dstp[sel]
        q_d = d_p // QROWS
        # per (node, quarter) counts
        need = np.zeros((NPC, NQ), np.int64)
        np.add.at(need, (s_l, q_d), 1)
        # self loop: its table row lives in quarter c//2
        self_q = np.full(NPC, c // 2, np.int64)
        need[np.arange(NPC), self_q] += 1
        dmax = need.max(axis=1)
        cls = np.full(NPC, -1, np.int64)
        for D in CLASS_D:
            m = (cls == -1) & (dmax <= D)
            cls[m] = D
        assert (cls > 0).all(), "degree exceeds largest class"
        cores.append(dict(c=c, s_l=s_l, d_p=d_p, q_d=q_d, need=need,
                          self_q=self_q, cls=cls))

    # shared chunk structure (same on all cores for SPMD); npp shrinks for
    # sparse classes so chunk rounding doesn't explode the slot count
    chunk_counts = {}
    npp_of = {}
    for D in CLASS_D:
        mx = max(int((cd['cls'] == D).sum()) for cd in cores)
        if not mx:
            chunk_counts[D] = 0
            npp_of[D] = 1
            continue
        npp = min(CLASS_NPP[D], -(-mx // 128))
        npp_of[D] = npp
        chunk_counts[D] = -(-mx // (128 * npp))

    # plan: one entry per class-chunk
    plan = []
    g0 = 0          # node-group offset
    col0 = 0        # slot-column offset (per quarter)
    for D in CLASS_D:
        npp = npp_of[D]
        for _ in range(chunk_counts[D]):
            plan.append(dict(D=D, npp=npp, cols=npp * D, g0=g0, col0=col0))
            g0 += npp
            col0 += npp * D
    qtot, ctot = g0, col0
    assert qtot > 0
    tot_idx = NQ * ctot * 128 * 8

    per_core_inputs = []
    perms = []
    for cd in cores:
        c, cls = cd['c'], cd['cls']
        # node -> (class-chunk, position)
        col_of_node = np.full(NPC, -1, np.int64)   # node's first slot column
        perm = np.full(qtot * 128, -1, np.int64)
        for D in CLASS_D:
            if chunk_counts[D] == 0:
                continue
            nodes = np.flatnonzero(cls == D)
            npp = npp_of[D]
            base_g = min(ch['g0'] for ch in plan if ch['D'] == D)
            base_col = min(ch['col0'] for ch in plan if ch['D'] == D)
            t = np.arange(len(nodes))
            chunk = t // (128 * npp)
            i_loc = t % (128 * npp)
            p = i_loc % 128
            j = i_loc // 128
            # groups are contiguous per class (chunks are consecutive)
            perm[(base_g + chunk * npp + j) * 128 + p] = nodes
            col_of_node[nodes] = base_col + chunk * npp * D + j * D
        node_p = np.full(NPC, -1, np.int64)
        m = np.flatnonzero(perm >= 0)
        node_p[perm[m]] = m % 128

        # slot matrix [4 quarters, ctot cols, 128 partitions] of quarter-local
        # rows, default = quarter zero row
        slots = np.full((NQ, ctot, 128), QZERO, np.int16)
        # self loops
        n_ar = np.arange(NPC)
        selfrow = (c * NODE_PAD + n_ar) % QROWS
        sq = cd['self_q']
        slots[sq, col_of_node[n_ar], node_p[n_ar]] = selfrow.astype(np.int16)
        # edges: rank within (node, quarter), offset +1 in self quarter
        s_l, d_p, q_d = cd['s_l'], cd['d_p'], cd['q_d']
        order = np.lexsort((q_d, s_l))
        ss, dd, qq = s_l[order], d_p[order], q_d[order]
        # rank of each sorted edge within its (node, quarter) run
        key_change = np.flatnonzero(
            (np.diff(ss) != 0) | (np.diff(qq) != 0)) + 1
        starts = np.zeros(len(ss), np.int64)
        starts[key_change] = key_change
        np.maximum.accumulate(starts, out=starts)
        rank = np.arange(len(ss)) - starts
        rank = rank + (qq == sq[ss])       # slot 0 = self loop there
        col = col_of_node[ss] + rank
        slots[qq, col, node_p[ss]] = (dd % QROWS).astype(np.int16)

        # reorder into the per-call [16, 64] wrapped layout:
        # input stream order: for chunk t, quarter q, octet o: block
        blocks = []
        for ch in plan:
            cols = ch['cols']
            pieces = [GMAXC] * (cols // GMAXC)
            if cols % GMAXC:
                pieces.append(cols % GMAXC)
            for q in range(NQ):
                coff = 0
                for pc in pieces:
                    cs = slots[q, ch['col0'] + coff:
                               ch['col0'] + coff + pc, :]
                    arr = cs.reshape(-1)                 # [pc*128] c-major
                    arr = arr.reshape(pc * 8, 16).T      # idx i at [i%16,i//16]
                    arr = np.tile(arr, (8, 1))           # [128, pc*8]
                    blocks.append(arr.reshape(-1))
                    coff += pc
        slot_stream = np.concatenate(blocks).astype(np.int16)
        assert slot_stream.size == tot_idx, (slot_stream.size, tot_idx)

        xT = np.zeros((DIN, NODE_PAD), np.float32)
        xT[:, :NPC] = x[c * NPC:(c + 1) * NPC].T
        per_core_inputs.append({"xT": xT, "slots": slot_stream})
        perms.append(perm)

    return plan, qtot, ctot, tot_idx, per_core_inputs, perms


def _build_program(plan, qtot, ctot, tot_idx):
    nc = bacc.Bacc("TRN2", target_bir_lowering=False, debug=False,
                   num_devices=NCORES)
    xT_d = nc.dram_tensor("xT", [DIN, NODE_PAD], F32, kind="ExternalInput")
    w_d = nc.dram_tensor("w1t", [DIN, DOUT], F32, kind="ExternalInput")
    b_d = nc.dram_tensor("bias", [DOUT, 1], F32, kind="ExternalInput")
    a2_d = nc.dram_tensor("a2", [DOUT, GF], F32, kind="ExternalInput")
    slots_d = nc.dram_tensor("slots", [tot_idx], I16, kind="ExternalInput")
    out_d = nc.dram_tensor("out", [qtot * 128, DOUT], F32,
                           kind="ExternalOutput")

    with tile.TileContext(nc) as tc:
        with tc.tile_pool(name="const", bufs=1) as constp, \
             tc.tile_pool(name="xp", bufs=3) as xp, \
             tc.tile_pool(name="work", bufs=3) as work, \
             tc.tile_pool(name="big", bufs=1) as big, \
             tc.tile_pool(name="gbuf", bufs=1) as gbuf, \
             tc.tile_pool(name="ibuf", bufs=2) as ibuf, \
             tc.tile_pool(name="ps", bufs=2, space="PSUM") as ps, \
             tc.tile_pool(name="pst", bufs=2, space="PSUM") as pst, \
             tc.tile_pool(name="dram", bufs=1, space="DRAM") as dram:

            w_sb = constp.tile([DIN, DOUT], F32)
            nc.sync.dma_start(out=w_sb[:], in_=w_d[:, :])
            b_sb = constp.tile([DOUT, 1], F32)
            nc.sync.dma_start(out=b_sb[:], in_=b_d[:, :])
            a2_sb = constp.tile([DOUT, GF], F32)
            nc.sync.dma_start(out=a2_sb[:], in_=a2_d[:, :])
            ident = constp.tile([128, 128], F32)
            make_identity(nc, ident[:])

            # ---- node phase: gT[f, n] = [p*h; p; 0] ----
            gT = big.tile([GF, NODE_PAD], F32, tag="gT")
            # feature 33 stays zero (padding); rows 32-33 zeroed up front
            # (partition slices must be 32-aligned)
            nc.vector.memset(gT[DOUT:GF, :], 0.0)
            for t in range(NCHUNKS):
                cs = slice(t * MMCHUNK, (t + 1) * MMCHUNK)
                xt = xp.tile([DIN, MMCHUNK], F32)
                nc.sync.dma_start(out=xt[:], in_=xT_d[:, cs])
                hps = ps.tile([DOUT, MMCHUNK], F32, space="PSUM")
                nc.tensor.matmul(hps[:], lhsT=w_sb[:], rhs=xt[:],
                                 start=True, stop=True)
                h_sb = work.tile([DOUT, MMCHUNK], F32, tag="h")
                nc.vector.tensor_tensor(
                    out=h_sb[:], in0=hps[:],
                    in1=b_sb[:].to_broadcast([DOUT, MMCHUNK]),
                    op=mybir.AluOpType.add)
                z_sb = work.tile([DOUT, MMCHUNK], F32, tag="z")
                nc.vector.scalar_tensor_tensor(
                    out=z_sb[:], in0=h_sb[:], scalar=SLOPE,
                    in1=h_sb[:], op0=mybir.AluOpType.mult,
                    op1=mybir.AluOpType.max)
                sps = ps.tile([GF, MMCHUNK], F32, space="PSUM", tag="s2")
                nc.tensor.matmul(sps[:], lhsT=a2_sb[:], rhs=z_sb[:],
                                 start=True, stop=True)
                p_sb = work.tile([GF, MMCHUNK], F32, tag="p")
                nc.scalar.activation(out=p_sb[:], in_=sps[:],
                                     func=mybir.ActivationFunctionType.Exp)
                nc.vector.tensor_tensor(
                    out=gT[0:DOUT, cs], in0=h_sb[:], in1=p_sb[0:DOUT, :],
                    op=mybir.AluOpType.mult)
                nc.vector.tensor_copy(out=gT[DOUT:DOUT + 1, cs],
                                      in_=p_sb[DOUT:DOUT + 1, :])
            # node ZLOCAL must be all zeros (pad-slot target)
            nc.vector.memset(gT[:, ZLOCAL:ZLOCAL + 1], 0.0)

            # ---- transpose gT -> node-major, cast to bf16 table ----
            ntile = NODE_PAD // 128
            g_sb = big.tile([128, ntile * GF], F32)
            for t in range(ntile):
                tp = pst.tile([128, GF], F32, space="PSUM")
                nc.tensor.transpose(
                    out=tp[:], in_=gT[:, t * 128:(t + 1) * 128],
                    identity=ident[:GF, :GF])
                nc.vector.tensor_copy(
                    out=g_sb[:, t * GF:(t + 1) * GF], in_=tp[:])

            g_loc = dram.tile([NODE_PAD, GSTRIDE], BF16)
            nc.gpsimd.dma_start(
                out=g_loc[:, 0:GF].rearrange("(t p) f -> p t f", p=128),
                in_=g_sb[:].rearrange("p (t f) -> p t f", f=GF))
            zfill = work.tile([128, GSTRIDE - GF], BF16, tag="zf")
            nc.vector.memset(zfill[:], 0.0)
            nc.sync.dma_start(
                out=g_loc[:, GF:GSTRIDE].rearrange("(t p) f -> p t f", p=128),
                in_=bass.AP(zfill[:].tensor, 0,
                            [[GSTRIDE - GF, 128], [0, NODE_PAD // 128],
                             [1, GSTRIDE - GF]]))
            g_full = dram.tile([NCORES * NODE_PAD, GSTRIDE], BF16,
                               addr_space="Shared")
            nc.gpsimd.collective_compute(
                "AllGather", mybir.AluOpType.bypass,
                ins=[g_loc[:].opt()], outs=[g_full[:].opt()],
                replica_groups=[list(range(NCORES))])

            # ---- edge phase ----
            outbig = big.tile([128, qtot * GF], F32, tag="gT")
            ob = outbig[:].rearrange("p (q f) -> p q f", f=GF)
            ioff = 0
            for ch in plan:
                D, npp, cols, g0 = ch['D'], ch['npp'], ch['cols'], ch['g0']
                B = gbuf.tile([128, NQ * cols * GF], BF16, tag="B")
                pieces = [GMAXC] * (cols // GMAXC)
                if cols % GMAXC:
                    pieces.append(cols % GMAXC)
                idx = ibuf.tile([128, NQ * cols * 8], I16, tag="idx")
                foff = 0
                for q in range(NQ):
                    qsl = g_full[q * QROWS:(q + 1) * QROWS, 0:GF]
                    coff = 0
                    for pc in pieces:
                        blk = pc * 8
                        nc.sync.dma_start(
                            out=idx[:, foff:foff + blk],
                            in_=bass.AP(slots_d[:].tensor, ioff,
                                        [[blk, 128], [1, blk]]))
                        lo = (q * cols + coff) * GF
                        dst = B[:, lo:lo + pc * GF].rearrange(
                            "p (c f) -> p c f", f=GF)
                        dma_gather_raw(nc, dst, qsl,
                                       idx[:, foff:foff + blk],
                                       pc * 128, GF, GSTRIDE)
                        ioff += 128 * blk
                        foff += blk
                        coff += pc
                # reduce over (slot d, quarter q): 5D AP, X=d then Y=q
                inap = B[:].rearrange("p (q j d f) -> p j f q d",
                                      q=NQ, j=npp, d=D, f=GF)
                nc.vector.reduce_sum(out=ob[:, g0:g0 + npp, :], in_=inap,
                                     axis=mybir.AxisListType.XY)

            # ---- normalize and write out ----
            dt_ = work.tile([128, qtot], F32, tag="den")
            nc.vector.tensor_scalar_add(out=dt_[:], in0=ob[:, :, DOUT],
                                        scalar1=1e-30)
            rec = work.tile([128, qtot], F32, tag="rec")
            nc.vector.reciprocal(out=rec[:], in_=dt_[:])
            nc.vector.tensor_tensor(
                out=ob[:, :, 0:DOUT], in0=ob[:, :, 0:DOUT],
                in1=rec[:].to_broadcast([128, qtot, DOUT]),
                op=mybir.AluOpType.mult)
            nc.sync.dma_start(
                out=out_d[:, :].rearrange("(q p) f -> p q f", p=128),
                in_=ob[:, :, 0:DOUT])

    nc.compile()
    return nc


class _Runner:
    """shard_map-jitted executor (mirrors bass2jax.run_bass_via_pjrt)."""

    def __init__(self, nc, n_cores):
        install_neuronx_cc_hook()
        self.n_cores = n_cores
        partition_name = (nc.partition_id_tensor.name
                          if nc.partition_id_tensor else None)
        in_names, out_names, out_avals, zero_outs = [], [], [], []
        for alloc in nc.m.functions[0].allocations:
            if not isinstance(alloc, mybir.MemoryLocationSet):
                continue
            name = alloc.memorylocations[0].name
            if alloc.kind == "ExternalInput":
                if name != partition_name:
                    in_names.append(name)
            elif alloc.kind == "ExternalOutput":
                out_names.append(name)
                shape = tuple(alloc.tensor_shape)
                dtype = mybir.dt.np(alloc.dtype)
                out_avals.append(jax.core.ShapedArray(shape, dtype))
                zero_outs.append(np.zeros(shape, dtype))
        self.in_names = in_names
        self.out_names = out_names
        self.out_avals = out_avals
        self.zero_outs = zero_outs
        n_params = len(in_names)
        self.n_params = n_params
        all_in = in_names + out_names
        if partition_name is not None:
            all_in.append(partition_name)
        donate = tuple(range(n_params, n_params + len(out_avals)))

        def _body(*args):
            operands = list(args)
            if partition_name is not None:
                operands.append(bass2jax.partition_id_tensor())
            outs = _bass_exec_p.bind(
                *operands, out_avals=tuple(out_avals),
                in_names=tuple(all_in), out_names=tuple(out_names),
                lowering_input_output_aliases=(),
                sim_require_finite=True, sim_require_nnan=True, nc=nc)
            return tuple(outs)

        devices = jax.devices()[:n_cores]
        mesh = Mesh(np.asarray(devices), ("core",))
        self._fn = jax.jit(
            shard_map(_body, mesh=mesh,
                      in_specs=(PartitionSpec("core"),) * (n_params +
                                                           len(out_avals)),
                      out_specs=(PartitionSpec("core"),) * len(out_names),
                      check_rep=False),
            donate_argnums=donate, keep_unused=True)

    def run(self, in_maps):
        per_core = [[np.asarray(m[n]) for n in self.in_names]
                    for m in in_maps]
        concat_in = [
            np.concatenate([per_core[c][i] for c in range(self.n_cores)],
                           axis=0)
            for i in range(self.n_params)
        ]
        concat_zeros = [
            np.zeros((self.n_cores * z.shape[0], *z.shape[1:]), z.dtype)
            for z in self.zero_outs
        ]
        out_arrs = self._fn(*concat_in, *concat_zeros)
        jax.block_until_ready(out_arrs)
        return [
            {name: np.asarray(out_arrs[i]).reshape(
                self.n_cores, *self.out_avals[i].shape)[c]
             for i, name in enumerate(self.out_names)}
            for c in range(self.n_cores)
        ]


_CACHE = {}


def _consts(W1_w, W1_b, a2_w):
    return {
        "w1t": np.ascontiguousarray(np.asarray(W1_w).T).astype(np.float32),
        "bias": np.asarray(W1_b).reshape(DOUT, 1).astype(np.float32),
        "a2": np.repeat(np.asarray(a2_w).reshape(DOUT, 1), GF,
                        axis=1).astype(np.float32),
    }


def _get_runner(plan, qtot, ctot, tot_idx):
    key = (tuple((ch['D'], ch['g0']) for ch in plan), qtot, ctot, tot_idx)
    if key not in _CACHE:
        nc = _build_program(plan, qtot, ctot, tot_idx)
        _CACHE[key] = (nc, _Runner(nc, NCORES))
    return _CACHE[key]


def kernel(x, edge_index, W1_w, W1_b, a1_w=None, a2_w=None):
    plan, qtot, ctot, tot_idx, per_core, perms = _host_shard(x, edge_index)
    nc, runner = _get_runner(plan, qtot, ctot, tot_idx)
    consts = _consts(W1_w, W1_b, a2_w)
    in_maps = [{**per_core[c], **consts} for c in range(NCORES)]
    results = runner.run(in_maps)
    out = np.empty((N, DOUT), np.float32)
    for c in range(NCORES):
        rows = results[c]["out"]
        perm = perms[c]
        valid = perm >= 0
        out[c * NPC + perm[valid]] = rows[valid]
    return out


# revision 14
# speedup vs baseline: 1.4550x; 1.1771x over previous
"""GATv2 attention head (single head) on 8 Trainium2 NeuronCores.

Math: with h = x @ W1^T + b, z = leaky_relu(h), s2 = z@a2, the GATv2 segment
softmax over src makes the s1[src] term cancel (constant per segment), so
with p = exp(s2):

    out[i] = sum_{j in N(i) ∪ {i}} p[j] * h[j] / sum_{j} p[j]

i.e. a weighted average of h rows with per-node weights p[j].  What remains
is one segment-sum over edges of gathered rows g[j] = [p[j]*h[j], p[j]].

Sharding: edges are bucketed by src range (12500 nodes per core).  Each core
computes g for its range, AllGathers the full g table (bf16, 256B row
stride), then gathers g[dst] for its edges with the SWDGE dma_gather ucode
op (int16 indices => the 102400-row table is split into 4 quarters of 25600
rows) and segment-sums with a strided DVE reduction.  To keep the reduce
static, every node gets D slots per quarter where D (its "class") is the
smallest allowed value >= max over quarters of its per-quarter edge count
(self loop included, placed in the least-loaded quarter); pad slots point
at an all-zero table row.  The 4 quarter partials for a node land at the
same (partition, group) position, so one 5D reduce sums over both the slot
and quarter axes at once.
"""
import sys
sys.path.insert(0, '/opt/trn_rl_repo')

import numpy as np
import jax
from jax.sharding import Mesh, PartitionSpec
from jax.experimental.shard_map import shard_map

import concourse.bass as bass
import concourse.bacc as bacc
import concourse.mybir as mybir
import concourse.tile as tile
from concourse.bass import exact_div
from concourse.masks import make_identity
from concourse import bass2jax
from concourse.bass2jax import _bass_exec_p, install_neuronx_cc_hook

F32 = mybir.dt.float32
BF16 = mybir.dt.bfloat16
I16 = mybir.dt.int16

NCORES = 8
N = 100000
DIN = 128
DOUT = 32
SLOPE = 0.2
NPC = N // NCORES            # nodes per core (12500)
NODE_PAD = 12800             # padded nodes per core
GF = DOUT + 2                # stored row: [p*h (32), p, 0]
GSTRIDE = 128                # bf16 elements per table row (256B)
NQ = 4                       # index-range quarters (int16 limit)
QROWS = NCORES * NODE_PAD // NQ   # 25600 rows per quarter
ZLOCAL = NODE_PAD - 1        # per-core always-zero g row
QZERO = NODE_PAD + ZLOCAL    # quarter-local zero row (25599), in every quarter
MMCHUNK = 512
NCHUNKS = NODE_PAD // MMCHUNK
GMAXC = 32                   # max slot columns (4096 idx) per dma_gather call

# degree classes: D slots per node per quarter; cols per chunk = npp*D must
# be a multiple of GCOLS(8)
CLASS_NPP = {2: 64, 4: 32, 6: 20, 8: 16, 10: 12, 12: 10, 14: 8, 16: 8,
             20: 6, 24: 5, 28: 4, 32: 4, 40: 3, 48: 2, 64: 2, 96: 1, 128: 1}
CLASS_D = sorted(CLASS_NPP)


def dma_gather_raw(nc, out_ap, in_ap, idxs_ap, num_idxs, elem_size,
                   elem_step, single_packet=False):
    """bass.dma_gather minus the elem%256 assert (only the row *stride*
    must be a 256B multiple for the ucode)."""
    assert idxs_ap.dtype == I16
    assert in_ap.dtype == out_ap.dtype
    assert in_ap.ap[0][0] == elem_step
    stride_bytes_256 = exact_div(elem_step * mybir.dt.size(in_ap.dtype), 256)
    eng = nc.gpsimd
    _in_ap = eng.lower_ap_dma(in_ap, for_custom_bir_dma=True)
    _idxs_ap = eng.lower_ap(idxs_ap)
    _out_ap = eng.lower_ap(out_ap)
    return eng.add_instruction(
        mybir.InstDMAGatherAnt(
            name=nc.get_next_instruction_name(),
            ins=[*_in_ap, _idxs_ap,
                 eng.lower_val_access(eng.to_reg(num_idxs))],
            outs=[_out_ap],
            transpose=False, num_idxs=num_idxs, elem_size=elem_size,
            stride_bytes_256=stride_bytes_256, gen_mode=0,
            single_packet=single_packet, queue_num=0, sbuf_tokens_per_rank=0,
            sbuf_free_dim_per_rank=0, sbuf_free_dim_pad_per_rank=0,
            sbuf_byte_offset=0))


def _host_shard(x, edge_index):
    """Bucket edges by src range; build per-core quarter slot indices."""
    src = np.asarray(edge_index[0]).astype(np.int64)
    dst = np.asarray(edge_index[1]).astype(np.int64)
    x = np.asarray(x)
    dstp = (dst // NPC) * NODE_PAD + (dst % NPC)   # padded-global row

    cores = []
    for c in range(NCORES):
        sel = np.flatnonzero(src // NPC == c)
        s_l = src[sel] - c * NPC
        d_p = dstp[sel]
        q_d = d_p // QROWS
        # per (node, quarter) counts
        need = np.zeros((NPC, NQ), np.int64)
        np.add.at(need, (s_l, q_d), 1)
        # self loop: its table row lives in quarter c//2
        self_q = np.full(NPC, c // 2, np.int64)
        need[np.arange(NPC), self_q] += 1
        dmax = need.max(axis=1)
        cls = np.full(NPC, -1, np.int64)
        for D in CLASS_D:
            m = (cls == -1) & (dmax <= D)
            cls[m] = D
        assert (cls > 0).all(), "degree exceeds largest class"
        cores.append(dict(c=c, s_l=s_l, d_p=d_p, q_d=q_d, need=need,
                          self_q=self_q, cls=cls))

    # shared chunk structure (same on all cores for SPMD); npp shrinks for
    # sparse classes so chunk rounding doesn't explode the slot count
    chunk_counts = {}
    npp_of = {}
    for D in CLASS_D:
        mx = max(int((cd['cls'] == D).sum()) for cd in cores)
        if not mx:
            chunk_counts[D] = 0
            npp_of[D] = 1
            continue
        npp = min(CLASS_NPP[D], -(-mx // 128))
        npp_of[D] = npp
        chunk_counts[D] = -(-mx // (128 * npp))

    # plan: one entry per class-chunk
    plan = []
    g0 = 0          # node-group offset
    col0 = 0        # slot-column offset (per quarter)
    for D in CLASS_D:
        npp = npp_of[D]
        for _ in range(chunk_counts[D]):
            plan.append(dict(D=D, npp=npp, cols=npp * D, g0=g0, col0=col0))
            g0 += npp
            col0 += npp * D
    qtot, ctot = g0, col0
    assert qtot > 0
    tot_idx = NQ * ctot * 128 * 8

    per_core_inputs = []
    perms = []
    for cd in cores:
        c, cls = cd['c'], cd['cls']
        # node -> (class-chunk, position)
        col_of_node = np.full(NPC, -1, np.int64)   # node's first slot column
        perm = np.full(qtot * 128, -1, np.int64)
        for D in CLASS_D:
            if chunk_counts[D] == 0:
                continue
            nodes = np.flatnonzero(cls == D)
            npp = npp_of[D]
            base_g = min(ch['g0'] for ch in plan if ch['D'] == D)
            base_col = min(ch['col0'] for ch in plan if ch['D'] == D)
            t = np.arange(len(nodes))
            chunk = t // (128 * npp)
            i_loc = t % (128 * npp)
            p = i_loc % 128
            j = i_loc // 128
            # groups are contiguous per class (chunks are consecutive)
            perm[(base_g + chunk * npp + j) * 128 + p] = nodes
            col_of_node[nodes] = base_col + chunk * npp * D + j * D
        node_p = np.full(NPC, -1, np.int64)
        m = np.flatnonzero(perm >= 0)
        node_p[perm[m]] = m % 128

        # slot matrix [4 quarters, ctot cols, 128 partitions] of quarter-local
        # rows, default = quarter zero row
        slots = np.full((NQ, ctot, 128), QZERO, np.int16)
        # self loops
        n_ar = np.arange(NPC)
        selfrow = (c * NODE_PAD + n_ar) % QROWS
        sq = cd['self_q']
        slots[sq, col_of_node[n_ar], node_p[n_ar]] = selfrow.astype(np.int16)
        # edges: rank within (node, quarter), offset +1 in self quarter
        s_l, d_p, q_d = cd['s_l'], cd['d_p'], cd['q_d']
        order = np.lexsort((q_d, s_l))
        ss, dd, qq = s_l[order], d_p[order], q_d[order]
        # rank of each sorted edge within its (node, quarter) run
        key_change = np.flatnonzero(
            (np.diff(ss) != 0) | (np.diff(qq) != 0)) + 1
        starts = np.zeros(len(ss), np.int64)
        starts[key_change] = key_change
        np.maximum.accumulate(starts, out=starts)
        rank = np.arange(len(ss)) - starts
        rank = rank + (qq == sq[ss])       # slot 0 = self loop there
        col = col_of_node[ss] + rank
        slots[qq, col, node_p[ss]] = (dd % QROWS).astype(np.int16)

        # reorder into the per-call [16, 64] wrapped layout:
        # input stream order: for chunk t, quarter q, octet o: block
        blocks = []
        for ch in plan:
            cols = ch['cols']
            pieces = [GMAXC] * (cols // GMAXC)
            if cols % GMAXC:
                pieces.append(cols % GMAXC)
            for q in range(NQ):
                coff = 0
                for pc in pieces:
                    cs = slots[q, ch['col0'] + coff:
                               ch['col0'] + coff + pc, :]
                    arr = cs.reshape(-1)                 # [pc*128] c-major
                    arr = arr.reshape(pc * 8, 16).T      # idx i at [i%16,i//16]
                    arr = np.tile(arr, (8, 1))           # [128, pc*8]
                    blocks.append(arr.reshape(-1))
                    coff += pc
        slot_stream = np.concatenate(blocks).astype(np.int16)
        assert slot_stream.size == tot_idx, (slot_stream.size, tot_idx)

        xT = np.zeros((DIN, NODE_PAD), np.float32)
        xT[:, :NPC] = x[c * NPC:(c + 1) * NPC].T
        per_core_inputs.append({"xT": xT, "slots": slot_stream})
        perms.append(perm)

    return plan, qtot, ctot, tot_idx, per_core_inputs, perms


def _build_program(plan, qtot, ctot, tot_idx):
    nc = bacc.Bacc("TRN2", target_bir_lowering=False, debug=False,
                   num_devices=NCORES)
    xT_d = nc.dram_tensor("xT", [DIN, NODE_PAD], F32, kind="ExternalInput")
    w_d = nc.dram_tensor("w1t", [DIN, DOUT], F32, kind="ExternalInput")
    b_d = nc.dram_tensor("bias", [DOUT, 1], F32, kind="ExternalInput")
    a2_d = nc.dram_tensor("a2", [DOUT, GF], F32, kind="ExternalInput")
    slots_d = nc.dram_tensor("slots", [tot_idx], I16, kind="ExternalInput")
    out_d = nc.dram_tensor("out", [qtot * 128, DOUT], F32,
                           kind="ExternalOutput")

    with tile.TileContext(nc) as tc:
        with tc.tile_pool(name="const", bufs=1) as constp, \
             tc.tile_pool(name="xp", bufs=3) as xp, \
             tc.tile_pool(name="work", bufs=3) as work, \
             tc.tile_pool(name="big", bufs=1) as big, \
             tc.tile_pool(name="gbuf", bufs=2) as gbuf, \
             tc.tile_pool(name="ibuf", bufs=2) as ibuf, \
             tc.tile_pool(name="ps", bufs=2, space="PSUM") as ps, \
             tc.tile_pool(name="pst", bufs=2, space="PSUM") as pst, \
             tc.tile_pool(name="dram", bufs=1, space="DRAM") as dram:

            w_sb = constp.tile([DIN, DOUT], F32)
            nc.sync.dma_start(out=w_sb[:], in_=w_d[:, :])
            b_sb = constp.tile([DOUT, 1], F32)
            nc.sync.dma_start(out=b_sb[:], in_=b_d[:, :])
            a2_sb = constp.tile([DOUT, GF], F32)
            nc.sync.dma_start(out=a2_sb[:], in_=a2_d[:, :])
            ident = constp.tile([128, 128], F32)
            make_identity(nc, ident[:])

            # ---- node phase: gT[f, n] = [p*h; p; 0] ----
            gT = big.tile([GF, NODE_PAD], F32, tag="gT")
            # feature 33 stays zero (padding); rows 32-33 zeroed up front
            # (partition slices must be 32-aligned)
            nc.vector.memset(gT[DOUT:GF, :], 0.0)
            for t in range(NCHUNKS):
                cs = slice(t * MMCHUNK, (t + 1) * MMCHUNK)
                xt = xp.tile([DIN, MMCHUNK], F32)
                nc.sync.dma_start(out=xt[:], in_=xT_d[:, cs])
                hps = ps.tile([DOUT, MMCHUNK], F32, space="PSUM")
                nc.tensor.matmul(hps[:], lhsT=w_sb[:], rhs=xt[:],
                                 start=True, stop=True)
                h_sb = work.tile([DOUT, MMCHUNK], F32, tag="h")
                nc.vector.tensor_tensor(
                    out=h_sb[:], in0=hps[:],
                    in1=b_sb[:].to_broadcast([DOUT, MMCHUNK]),
                    op=mybir.AluOpType.add)
                z_sb = work.tile([DOUT, MMCHUNK], F32, tag="z")
                nc.vector.scalar_tensor_tensor(
                    out=z_sb[:], in0=h_sb[:], scalar=SLOPE,
                    in1=h_sb[:], op0=mybir.AluOpType.mult,
                    op1=mybir.AluOpType.max)
                sps = ps.tile([GF, MMCHUNK], F32, space="PSUM", tag="s2")
                nc.tensor.matmul(sps[:], lhsT=a2_sb[:], rhs=z_sb[:],
                                 start=True, stop=True)
                p_sb = work.tile([GF, MMCHUNK], F32, tag="p")
                nc.scalar.activation(out=p_sb[:], in_=sps[:],
                                     func=mybir.ActivationFunctionType.Exp)
                nc.vector.tensor_tensor(
                    out=gT[0:DOUT, cs], in0=h_sb[:], in1=p_sb[0:DOUT, :],
                    op=mybir.AluOpType.mult)
                nc.vector.tensor_copy(out=gT[DOUT:DOUT + 1, cs],
                                      in_=p_sb[DOUT:DOUT + 1, :])
            # node ZLOCAL must be all zeros (pad-slot target)
            nc.vector.memset(gT[:, ZLOCAL:ZLOCAL + 1], 0.0)

            # ---- transpose gT -> node-major, cast to bf16 table ----
            ntile = NODE_PAD // 128
            g_sb = big.tile([128, ntile * GF], F32)
            for t in range(ntile):
                tp = pst.tile([128, GF], F32, space="PSUM")
                nc.tensor.transpose(
                    out=tp[:], in_=gT[:, t * 128:(t + 1) * 128],
                    identity=ident[:GF, :GF])
                nc.vector.tensor_copy(
                    out=g_sb[:, t * GF:(t + 1) * GF], in_=tp[:])

            g_loc = dram.tile([NODE_PAD, GSTRIDE], BF16)
            nc.gpsimd.dma_start(
                out=g_loc[:, 0:GF].rearrange("(t p) f -> p t f", p=128),
                in_=g_sb[:].rearrange("p (t f) -> p t f", f=GF))
            zfill = work.tile([128, GSTRIDE - GF], BF16, tag="zf")
            nc.vector.memset(zfill[:], 0.0)
            nc.sync.dma_start(
                out=g_loc[:, GF:GSTRIDE].rearrange("(t p) f -> p t f", p=128),
                in_=bass.AP(zfill[:].tensor, 0,
                            [[GSTRIDE - GF, 128], [0, NODE_PAD // 128],
                             [1, GSTRIDE - GF]]))
            g_full = dram.tile([NCORES * NODE_PAD, GSTRIDE], BF16,
                               addr_space="Shared")
            nc.gpsimd.collective_compute(
                "AllGather", mybir.AluOpType.bypass,
                ins=[g_loc[:].opt()], outs=[g_full[:].opt()],
                replica_groups=[list(range(NCORES))])

            # ---- edge phase ----
            outbig = big.tile([128, qtot * GF], F32, tag="gT")
            ob = outbig[:].rearrange("p (q f) -> p q f", f=GF)
            ioff = 0
            for ch in plan:
                D, npp, cols, g0 = ch['D'], ch['npp'], ch['cols'], ch['g0']
                B = gbuf.tile([128, NQ * cols * GF], BF16, tag="B")
                pieces = [GMAXC] * (cols // GMAXC)
                if cols % GMAXC:
                    pieces.append(cols % GMAXC)
                idx = ibuf.tile([128, NQ * cols * 8], I16, tag="idx")
                foff = 0
                for q in range(NQ):
                    qsl = g_full[q * QROWS:(q + 1) * QROWS, 0:GF]
                    coff = 0
                    for pc in pieces:
                        blk = pc * 8
                        nc.sync.dma_start(
                            out=idx[:, foff:foff + blk],
                            in_=bass.AP(slots_d[:].tensor, ioff,
                                        [[blk, 128], [1, blk]]))
                        lo = (q * cols + coff) * GF
                        dst = B[:, lo:lo + pc * GF].rearrange(
                            "p (c f) -> p c f", f=GF)
                        dma_gather_raw(nc, dst, qsl,
                                       idx[:, foff:foff + blk],
                                       pc * 128, GF, GSTRIDE)
                        ioff += 128 * blk
                        foff += blk
                        coff += pc
                # reduce over (slot d, quarter q): 5D AP, X=d then Y=q
                inap = B[:].rearrange("p (q j d f) -> p j f q d",
                                      q=NQ, j=npp, d=D, f=GF)
                nc.vector.reduce_sum(out=ob[:, g0:g0 + npp, :], in_=inap,
                                     axis=mybir.AxisListType.XY)

            # ---- normalize and write out ----
            dt_ = work.tile([128, qtot], F32, tag="den")
            nc.vector.tensor_scalar_add(out=dt_[:], in0=ob[:, :, DOUT],
                                        scalar1=1e-30)
            rec = work.tile([128, qtot], F32, tag="rec")
            nc.vector.reciprocal(out=rec[:], in_=dt_[:])
            nc.vector.tensor_tensor(
                out=ob[:, :, 0:DOUT], in0=ob[:, :, 0:DOUT],
                in1=rec[:].to_broadcast([128, qtot, DOUT]),
                op=mybir.AluOpType.mult)
            nc.sync.dma_start(
                out=out_d[:, :].rearrange("(q p) f -> p q f", p=128),
                in_=ob[:, :, 0:DOUT])

    nc.compile()
    return nc


class _Runner:
    """shard_map-jitted executor (mirrors bass2jax.run_bass_via_pjrt)."""

    def __init__(self, nc, n_cores):
        install_neuronx_cc_hook()
        self.n_cores = n_cores
        partition_name = (nc.partition_id_tensor.name
                          if nc.partition_id_tensor else None)
        in_names, out_names, out_avals, zero_outs = [], [], [], []
        for alloc in nc.m.functions[0].allocations:
            if not isinstance(alloc, mybir.MemoryLocationSet):
                continue
            name = alloc.memorylocations[0].name
            if alloc.kind == "ExternalInput":
                if name != partition_name:
                    in_names.append(name)
            elif alloc.kind == "ExternalOutput":
                out_names.append(name)
                shape = tuple(alloc.tensor_shape)
                dtype = mybir.dt.np(alloc.dtype)
                out_avals.append(jax.core.ShapedArray(shape, dtype))
                zero_outs.append(np.zeros(shape, dtype))
        self.in_names = in_names
        self.out_names = out_names
        self.out_avals = out_avals
        self.zero_outs = zero_outs
        n_params = len(in_names)
        self.n_params = n_params
        all_in = in_names + out_names
        if partition_name is not None:
            all_in.append(partition_name)
        donate = tuple(range(n_params, n_params + len(out_avals)))

        def _body(*args):
            operands = list(args)
            if partition_name is not None:
                operands.append(bass2jax.partition_id_tensor())
            outs = _bass_exec_p.bind(
                *operands, out_avals=tuple(out_avals),
                in_names=tuple(all_in), out_names=tuple(out_names),
                lowering_input_output_aliases=(),
                sim_require_finite=True, sim_require_nnan=True, nc=nc)
            return tuple(outs)

        devices = jax.devices()[:n_cores]
        mesh = Mesh(np.asarray(devices), ("core",))
        self._fn = jax.jit(
            shard_map(_body, mesh=mesh,
                      in_specs=(PartitionSpec("core"),) * (n_params +
                                                           len(out_avals)),
                      out_specs=(PartitionSpec("core"),) * len(out_names),
                      check_rep=False),
            donate_argnums=donate, keep_unused=True)

    def run(self, in_maps):
        per_core = [[np.asarray(m[n]) for n in self.in_names]
                    for m in in_maps]
        concat_in = [
            np.concatenate([per_core[c][i] for c in range(self.n_cores)],
                           axis=0)
            for i in range(self.n_params)
        ]
        concat_zeros = [
            np.zeros((self.n_cores * z.shape[0], *z.shape[1:]), z.dtype)
            for z in self.zero_outs
        ]
        out_arrs = self._fn(*concat_in, *concat_zeros)
        jax.block_until_ready(out_arrs)
        return [
            {name: np.asarray(out_arrs[i]).reshape(
                self.n_cores, *self.out_avals[i].shape)[c]
             for i, name in enumerate(self.out_names)}
            for c in range(self.n_cores)
        ]


_CACHE = {}


def _consts(W1_w, W1_b, a2_w):
    return {
        "w1t": np.ascontiguousarray(np.asarray(W1_w).T).astype(np.float32),
        "bias": np.asarray(W1_b).reshape(DOUT, 1).astype(np.float32),
        "a2": np.repeat(np.asarray(a2_w).reshape(DOUT, 1), GF,
                        axis=1).astype(np.float32),
    }


def _get_runner(plan, qtot, ctot, tot_idx):
    key = (tuple((ch['D'], ch['g0']) for ch in plan), qtot, ctot, tot_idx)
    if key not in _CACHE:
        nc = _build_program(plan, qtot, ctot, tot_idx)
        _CACHE[key] = (nc, _Runner(nc, NCORES))
    return _CACHE[key]


def kernel(x, edge_index, W1_w, W1_b, a1_w=None, a2_w=None):
    plan, qtot, ctot, tot_idx, per_core, perms = _host_shard(x, edge_index)
    nc, runner = _get_runner(plan, qtot, ctot, tot_idx)
    consts = _consts(W1_w, W1_b, a2_w)
    in_maps = [{**per_core[c], **consts} for c in range(NCORES)]
    results = runner.run(in_maps)
    out = np.empty((N, DOUT), np.float32)
    for c in range(NCORES):
        rows = results[c]["out"]
        perm = perms[c]
        valid = perm >= 0
        out[c * NPC + perm[valid]] = rows[valid]
    return out
